# revision 3
# baseline (speedup 1.0000x reference)
"""AdapterFusionBlock Trainium2 kernel v2: 8-way batch-parallel, one sample/core.

Restructured vs v1: LN outputs stay in SBUF (PE transposes, no DMA transposes),
shuffle-adapter folded into the QKV chunk loop, attention computes transposed
scores with rel-pos bias via small per-group matmuls (no DRAM bounce chain),
softmax normalization deferred to per-partition scaling, fp8 DoubleRow matmuls
for qkv/adapter/proj/PV (MLP stays bf16 for accuracy).
"""
import sys
sys.path.insert(0, '/opt/trn_rl_repo')
import numpy as np
import ml_dtypes
import concourse.bass as bass
import concourse.mybir as mybir
import concourse.tile as tile
from concourse import bacc
from concourse.bass_utils import run_bass_kernel_spmd
from concourse.masks import make_identity

FP32 = mybir.dt.float32
BF16 = mybir.dt.bfloat16
FP8 = mybir.dt.float8e4
AF = mybir.ActivationFunctionType
ALU = mybir.AluOpType
DR = mybir.MatmulPerfMode.DoubleRow

DIM = 768; NH = 12; HD = 64; WS = 16; B = 8; H = 64; W = 64
MLPD = 4 * DIM; AD = 3 * DIM // 4; ADP = 640  # padded adapter dim
HID = DIM // 2
BLOCK_SCALE = 0.5; EPS = 1e-5
T = H * W                  # 4096 tokens per core
NWIN = (H // WS) * (W // WS)   # 16 windows
NT = WS * WS               # 256 tokens per window
CH = 512                   # token chunk for GEMM phases
NCH = T // CH              # 8
SCALE = HD ** -0.5         # 0.125
NTILES = T // 128          # 32

_BF = ml_dtypes.bfloat16
_F8 = ml_dtypes.float8_e4m3


def _bf16(x):
    return np.ascontiguousarray(np.asarray(x, np.float32).astype(_BF))


def _fp8(x, s):
    return np.ascontiguousarray(
        np.clip(np.asarray(x, np.float32) * s, -240, 240).astype(_F8))


def _pow2_scale(x):
    amax = float(np.abs(x).max())
    if amax <= 0:
        return 1.0
    return float(2.0 ** np.floor(np.log2(192.0 / amax)))


def _col_tiles(v):
    """[n*128] -> [128, n] column layout."""
    n = v.shape[0] // 128
    return np.ascontiguousarray(v.reshape(n, 128).T.astype(np.float32))


def _ap(t, coloff, dims):
    """SBUF/PSUM AP: partition dim (full tile) + custom free dims."""
    return bass.AP(tensor=t.tensor, offset=t[:, :].offset + coloff,
                   ap=[[t.tensor.shape[1], t.shape[0]]] + dims)


def build_graph(scales):
    S_QKV, S_A1, S_A2, S_WP = scales
    nc = bacc.Bacc()
    P = 128

    # ---------------- DRAM parameters ----------------
    x_in = nc.declare_dram_parameter("x", [T, DIM], FP32, isOutput=False)
    wqkv8 = nc.declare_dram_parameter("wqkv8", [DIM, 3 * DIM], FP8, isOutput=False)
    a1w8 = nc.declare_dram_parameter("a1w8", [3 * DIM, ADP], FP8, isOutput=False)
    a2w8 = nc.declare_dram_parameter("a2w8", [512, 3 * DIM], FP8, isOutput=False)
    a2wt = nc.declare_dram_parameter("a2wt", [P, 3 * DIM], BF16, isOutput=False)
    wp8 = nc.declare_dram_parameter("wp8", [DIM, DIM], FP8, isOutput=False)
    wm1 = nc.declare_dram_parameter("wm1", [DIM, MLPD], BF16, isOutput=False)
    wm2 = nc.declare_dram_parameter("wm2", [MLPD, DIM], BF16, isOutput=False)
    rhcat = nc.declare_dram_parameter("rhcat", [HD, 512], BF16, isOutput=False)
    rwcat = nc.declare_dram_parameter("rwcat", [HD, 256], BF16, isOutput=False)
    indic48 = nc.declare_dram_parameter("indic48", [48, NT], BF16, isOutput=False)
    bqkv_c = nc.declare_dram_parameter("bqkv_c", [P, 18], FP32, isOutput=False)
    ba1_c = nc.declare_dram_parameter("ba1_c", [P, 5], FP32, isOutput=False)
    ba2_c = nc.declare_dram_parameter("ba2_c", [P, 18], FP32, isOutput=False)
    bm1_c = nc.declare_dram_parameter("bm1_c", [P, 24], FP32, isOutput=False)
    bp_r = nc.declare_dram_parameter("bp_r", [1, DIM], BF16, isOutput=False)  # x S_WP
    bm2_r = nc.declare_dram_parameter("bm2_r", [1, DIM], BF16, isOutput=False)
    w1_c = nc.declare_dram_parameter("w1_c", [P, 6], FP32, isOutput=False)
    b1_c = nc.declare_dram_parameter("b1_c", [P, 6], FP32, isOutput=False)
    w1_r = nc.declare_dram_parameter("w1_r", [1, DIM], FP32, isOutput=False)
    b1_r = nc.declare_dram_parameter("b1_r", [1, DIM], FP32, isOutput=False)
    cw_r = nc.declare_dram_parameter("cw_r", [1, HID], FP32, isOutput=False)
    cb_r = nc.declare_dram_parameter("cb_r", [1, HID], FP32, isOutput=False)
    sw_r = nc.declare_dram_parameter("sw_r", [1, HID], FP32, isOutput=False)
    sb_r = nc.declare_dram_parameter("sb_r", [1, HID], FP32, isOutput=False)
    out_ext = nc.declare_dram_parameter("out", [T, DIM], FP32, isOutput=True)

    # ---------------- DRAM scratch ----------------
    q_d = nc.dram_tensor("q_d", [DIM, T], BF16)      # feature-major (h*64+hd)
    k_d = nc.dram_tensor("k_d", [DIM, T], BF16)
    v8_d = nc.dram_tensor("v8_d", [T, 780], FP8)     # token-major fp8, 65/head (v|1)
    ad_d = nc.dram_tensor("ad_d", [T, DIM], BF16)    # token-major, pre-shuffled
    xm_d = nc.dram_tensor("xm_d", [T, DIM], BF16)    # token-major residual

    with tile.TileContext(nc) as tc:
        with tc.tile_pool(name="const", bufs=1) as const, \
             tc.tile_pool(name="stats", bufs=1) as stats:
            nrm_ctx = tc.tile_pool(name="nrm", bufs=1)
            nrm = nrm_ctx.__enter__()
            ident = const.tile([P, P], BF16)
            make_identity(nc, ident[:, :])
            ones1 = const.tile([1, P], BF16)
            nc.vector.memset(ones1[:, :], 1.0)
            ones128 = const.tile([P, 1], BF16)
            nc.vector.memset(ones128[:, :], 1.0)
            ones8 = const.tile([P, 2], FP8)
            nc.vector.memset(ones8[:, :], 1.0)
            eps_col = const.tile([P, 1], FP32)
            nc.vector.memset(eps_col[:, :], EPS)
            rh_sb = const.tile([HD, 512], BF16)
            nc.sync.dma_start(out=rh_sb[:, :], in_=rhcat[:, :])
            rw_sb = const.tile([HD, 256], BF16)
            nc.sync.dma_start(out=rw_sb[:, :], in_=rwcat[:, :])
            ind48_sb = const.tile([48, NT], BF16)
            nc.sync.dma_start(out=ind48_sb[:, :], in_=indic48[:, :])
            bp_sb = const.tile([1, DIM], BF16)
            nc.sync.dma_start(out=bp_sb[:, :], in_=bp_r[:, :])
            bm2_sb = const.tile([1, DIM], BF16)
            nc.sync.dma_start(out=bm2_sb[:, :], in_=bm2_r[:, :])
            bqkv_sb = const.tile([P, 18], FP32)
            nc.sync.dma_start(out=bqkv_sb[:, :], in_=bqkv_c[:, :])
            ba1_sb = const.tile([P, 5], FP32)
            nc.sync.dma_start(out=ba1_sb[:, :], in_=ba1_c[:, :])
            ba2_sb = const.tile([P, 18], FP32)
            nc.sync.dma_start(out=ba2_sb[:, :], in_=ba2_c[:, :])
            bm1_sb = const.tile([P, 24], FP32)
            nc.sync.dma_start(out=bm1_sb[:, :], in_=bm1_c[:, :])
            w1c_sb = const.tile([P, 6], FP32)
            nc.sync.dma_start(out=w1c_sb[:, :], in_=w1_c[:, :])
            b1c_sb = const.tile([P, 6], FP32)
            nc.sync.dma_start(out=b1c_sb[:, :], in_=b1_c[:, :])
            w1r_sb = const.tile([1, DIM], FP32)
            nc.sync.dma_start(out=w1r_sb[:, :], in_=w1_r[:, :])
            b1r_sb = const.tile([1, DIM], FP32)
            nc.sync.dma_start(out=b1r_sb[:, :], in_=b1_r[:, :])
            cw_sb = const.tile([1, HID], FP32)
            nc.sync.dma_start(out=cw_sb[:, :], in_=cw_r[:, :])
            cb_sb = const.tile([1, HID], FP32)
            nc.sync.dma_start(out=cb_sb[:, :], in_=cb_r[:, :])
            sw_sb = const.tile([1, HID], FP32)
            nc.sync.dma_start(out=sw_sb[:, :], in_=sw_r[:, :])
            sb_sb = const.tile([1, HID], FP32)
            nc.sync.dma_start(out=sb_sb[:, :], in_=sb_r[:, :])

            # persistent activation buffers (freed after phase 2a)
            normT = [nrm.tile([P, T], BF16, name=f"normT{k}") for k in range(6)]

            # ============ PHASE 1: LN1 -> normT/normT8 + adapter stats ============
            with tc.tile_pool(name="p1", bufs=4) as p1, \
                 tc.tile_pool(name="p1ps", bufs=1, space="PSUM") as p1ps, \
                 tc.tile_pool(name="p1tp", bufs=2, space="PSUM") as p1tp:
                sum_ps = p1ps.tile([1, 1024], FP32)
                sq_ps = p1ps.tile([1, 1024], FP32)
                for t in range(NTILES):
                    xt = p1.tile([P, DIM], FP32, tag="xt", bufs=4)
                    nc.sync.dma_start(out=xt[:, :], in_=x_in[t * P:(t + 1) * P, :])
                    sm = p1.tile([P, 1], FP32, tag="sm", bufs=4)
                    nc.vector.tensor_reduce(sm[:, :], xt[:, :],
                                            axis=mybir.AxisListType.X, op=ALU.add)
                    scr = p1.tile([P, DIM], BF16, tag="scr", bufs=4)
                    sq = p1.tile([P, 1], FP32, tag="sq", bufs=4)
                    nc.scalar.activation(scr[:, :], xt[:, :], AF.Square,
                                         accum_out=sq[:, :])
                    mean = p1.tile([P, 1], FP32, tag="mean", bufs=4)
                    nc.vector.tensor_scalar(mean[:, :], sm[:, :], 1.0 / DIM, None, op0=ALU.mult)
                    var = p1.tile([P, 1], FP32, tag="var", bufs=4)
                    nc.vector.tensor_scalar(var[:, :], sq[:, :], 1.0 / DIM, None, op0=ALU.mult)
                    m2c = p1.tile([P, 1], FP32, tag="m2c", bufs=4)
                    nc.vector.tensor_tensor(m2c[:, :], mean[:, :], mean[:, :], op=ALU.mult)
                    nc.vector.tensor_tensor(var[:, :], var[:, :], m2c[:, :], op=ALU.subtract)
                    sdv = p1.tile([P, 1], FP32, tag="sdv", bufs=4)
                    nc.scalar.activation(sdv[:, :], var[:, :], AF.Sqrt, bias=eps_col[:, :])
                    rstd = p1.tile([P, 1], FP32, tag="rstd", bufs=4)
                    nc.vector.reciprocal(rstd[:, :], sdv[:, :])
                    nt = p1.tile([P, DIM], BF16, tag="nt", bufs=4)
                    nc.vector.tensor_scalar(nt[:, :], xt[:, :], mean[:, :],
                                            rstd[:, :], op0=ALU.subtract, op1=ALU.mult)
                    nsq = p1.tile([P, DIM], BF16, tag="nsq", bufs=4)
                    nc.scalar.activation(nsq[:, :], nt[:, :], AF.Square)
                    for sl in (slice(0, 512), slice(512, 768)):
                        nc.tensor.matmul(sum_ps[:, sl], ones128[:, :], nt[:, sl],
                                         start=(t == 0), stop=(t == NTILES - 1))
                        nc.tensor.matmul(sq_ps[:, sl], ones128[:, :], nsq[:, sl],
                                         start=(t == 0), stop=(t == NTILES - 1))
                    # transposes -> normT (bf16) and normT8 (fp8)
                    ntT_ps = p1tp.tile([P, DIM], BF16, tag="ntT")
                    for k in range(6):
                        nc.tensor.transpose(ntT_ps[:, k * P:(k + 1) * P],
                                            nt[:, k * P:(k + 1) * P], ident[:, :])
                    for k in range(6):
                        nc.scalar.copy(normT[k][:, t * P:(t + 1) * P],
                                       ntT_ps[:, k * P:(k + 1) * P])
                # adapter per-channel factors (same math as v1)
                Mn = stats.tile([1, DIM], FP32)
                nc.vector.tensor_scalar(Mn[:, :], sum_ps[:, 0:DIM], 1.0 / T, None, op0=ALU.mult)
                Sq = stats.tile([1, DIM], FP32)
                nc.vector.tensor_scalar(Sq[:, :], sq_ps[:, 0:DIM], 1.0 / T, None, op0=ALU.mult)
                mch = stats.tile([1, HID], FP32)
                nc.vector.tensor_tensor(mch[:, :], w1r_sb[:, 0:HID], Mn[:, 0:HID], op=ALU.mult)
                nc.vector.tensor_tensor(mch[:, :], mch[:, :], b1r_sb[:, 0:HID], op=ALU.add)
                sig_in = stats.tile([1, HID], FP32)
                nc.vector.tensor_tensor(sig_in[:, :], cw_sb[:, :], mch[:, :], op=ALU.mult)
                nc.vector.tensor_tensor(sig_in[:, :], sig_in[:, :], cb_sb[:, :], op=ALU.add)
                s0 = stats.tile([1, HID], FP32)
                nc.scalar.activation(s0[:, :], sig_in[:, :], AF.Sigmoid)
                g0 = stats.tile([1, HID], FP32)
                nc.vector.tensor_tensor(g0[:, :], w1r_sb[:, 0:HID], s0[:, :], op=ALU.mult)
                h0 = stats.tile([1, HID], FP32)
                nc.vector.tensor_tensor(h0[:, :], b1r_sb[:, 0:HID], s0[:, :], op=ALU.mult)
                u = stats.tile([1, HID], FP32)
                nc.vector.tensor_tensor(u[:, :], w1r_sb[:, HID:DIM], Mn[:, HID:DIM], op=ALU.mult)
                nc.vector.tensor_tensor(u[:, :], u[:, :], b1r_sb[:, HID:DIM], op=ALU.add)
                mu = stats.tile([1, 1], FP32)
                nc.vector.tensor_reduce(mu[:, :], u[:, :], axis=mybir.AxisListType.X, op=ALU.add)
                nc.vector.tensor_scalar(mu[:, :], mu[:, :], 1.0 / HID, None, op0=ALU.mult)
                e1 = stats.tile([1, HID], FP32)
                nc.vector.tensor_tensor(e1[:, :], w1r_sb[:, HID:DIM], w1r_sb[:, HID:DIM], op=ALU.mult)
                nc.vector.tensor_tensor(e1[:, :], e1[:, :], Sq[:, HID:DIM], op=ALU.mult)
                e2 = stats.tile([1, HID], FP32)
                nc.vector.tensor_tensor(e2[:, :], w1r_sb[:, HID:DIM], b1r_sb[:, HID:DIM], op=ALU.mult)
                nc.vector.tensor_tensor(e2[:, :], e2[:, :], Mn[:, HID:DIM], op=ALU.mult)
                nc.vector.tensor_scalar(e2[:, :], e2[:, :], 2.0, None, op0=ALU.mult)
                nc.vector.tensor_tensor(e1[:, :], e1[:, :], e2[:, :], op=ALU.add)
                e3 = stats.tile([1, HID], FP32)
                nc.vector.tensor_tensor(e3[:, :], b1r_sb[:, HID:DIM], b1r_sb[:, HID:DIM], op=ALU.mult)
                nc.vector.tensor_tensor(e1[:, :], e1[:, :], e3[:, :], op=ALU.add)
                E2 = stats.tile([1, 1], FP32)
                nc.vector.tensor_reduce(E2[:, :], e1[:, :], axis=mybir.AxisListType.X, op=ALU.add)
                nc.vector.tensor_scalar(E2[:, :], E2[:, :], 1.0 / HID, None, op0=ALU.mult)
                mu2 = stats.tile([1, 1], FP32)
                nc.vector.tensor_tensor(mu2[:, :], mu[:, :], mu[:, :], op=ALU.mult)
                nc.vector.tensor_tensor(E2[:, :], E2[:, :], mu2[:, :], op=ALU.subtract)
                rv = stats.tile([1, 1], FP32)
                nc.scalar.activation(rv[:, :], E2[:, :], AF.Sqrt, bias=eps_col[0:1, :])
                nc.vector.reciprocal(rv[:, :], rv[:, :])
                Pv = stats.tile([1, HID], FP32)
                nc.vector.tensor_tensor(Pv[:, :], sw_sb[:, :], w1r_sb[:, HID:DIM], op=ALU.mult)
                nc.vector.tensor_scalar(Pv[:, :], Pv[:, :], rv[:, :], None, op0=ALU.mult)
                Qv = stats.tile([1, HID], FP32)
                nc.vector.tensor_scalar(Qv[:, :], b1r_sb[:, HID:DIM], mu[:, :], None, op0=ALU.subtract)
                nc.vector.tensor_tensor(Qv[:, :], Qv[:, :], sw_sb[:, :], op=ALU.mult)
                nc.vector.tensor_scalar(Qv[:, :], Qv[:, :], rv[:, :], None, op0=ALU.mult)
                nc.vector.tensor_tensor(Qv[:, :], Qv[:, :], sb_sb[:, :], op=ALU.add)
                vec_d = nc.dram_tensor("vec_d", [4, HID], FP32)
                nc.sync.dma_start(out=vec_d[0:1, :], in_=g0[0:1, :])
                nc.sync.dma_start(out=vec_d[1:2, :], in_=h0[0:1, :])
                nc.sync.dma_start(out=vec_d[2:3, :], in_=Pv[0:1, :])
                nc.sync.dma_start(out=vec_d[3:4, :], in_=Qv[0:1, :])
                g0c = stats.tile([P, 3], FP32); h0c = stats.tile([P, 3], FP32)
                Pc = stats.tile([P, 3], FP32); Qc = stats.tile([P, 3], FP32)
                for dst, row in ((g0c, 0), (h0c, 1), (Pc, 2), (Qc, 3)):
                    for kk in range(3):
                        src = bass.AP(tensor=vec_d[:, :].tensor,
                                      offset=row * HID + kk * P,
                                      ap=[[1, P], [1, 1]])
                        nc.sync.dma_start(out=dst[:, kk:kk + 1], in_=src)

            # ============ PHASE 2a: qkv + adapter + shuffle + v/ad transposes ====
            with tc.tile_pool(name="w2a", bufs=1) as w2a, \
                 tc.tile_pool(name="p2a", bufs=2) as p2a, \
                 tc.tile_pool(name="ps2a", bufs=4, space="PSUM") as ps2a, \
                 tc.tile_pool(name="pst2a", bufs=2, space="PSUM") as pst2a:
                wqkv_sb = w2a.tile([P, 6 * 2304], FP8)
                for k in range(6):
                    o0 = (k // 2) * 4608 + (k % 2) * 2304
                    nc.scalar.dma_start(out=wqkv_sb[:, o0:o0 + 2304],
                                        in_=wqkv8[k * P:(k + 1) * P, :])
                a1_sb = w2a.tile([P, 18 * ADP], FP8)
                for k in range(18):
                    o0 = (k // 2) * 2 * ADP + (k % 2) * ADP
                    nc.scalar.dma_start(out=a1_sb[:, o0:o0 + ADP],
                                        in_=a1w8[k * P:(k + 1) * P, :])
                a2_sb = w2a.tile([P, 4 * 2304], FP8)
                for k in range(4):
                    o0 = (k // 2) * 4608 + (k % 2) * 2304
                    nc.scalar.dma_start(out=a2_sb[:, o0:o0 + 2304],
                                        in_=a2w8[k * P:(k + 1) * P, :])
                a2t_sb = w2a.tile([P, 2304], BF16)
                nc.scalar.dma_start(out=a2t_sb[:, :], in_=a2wt[:, :])

                def qkv_stage(c):
                    csl = slice(c * CH, (c + 1) * CH)
                    # cast normT chunk to fp8 interleaved pairs
                    qn8 = p2a.tile([P, 6 * CH], FP8, tag="qn8", bufs=2)
                    for k in range(6):
                        c8 = (k // 2) * 2 * CH + (k % 2) * CH
                        nc.vector.tensor_scalar(qn8[:, c8:c8 + CH], normT[k][:, csl],
                                                1.0, None, op0=ALU.mult)
                    qkvT8 = p2a.tile([P, 18 * CH], FP8, tag="qkvT8", bufs=2)
                    for m in range(18):
                        ps = ps2a.tile([P, CH], FP32, tag="mm")
                        for kp in range(3):
                            lhsT = _ap(wqkv_sb, kp * 4608 + m * P, [[2304, 2], [1, P]])
                            rhs = _ap(qn8, kp * 2 * CH, [[CH, 2], [1, CH]])
                            nc.tensor.matmul(ps[:, :], lhsT, rhs, start=(kp == 0),
                                             stop=(kp == 2), perf_mode=DR)
                        c8 = (m // 2) * 2 * CH + (m % 2) * CH
                        nc.scalar.activation(qkvT8[:, c8:c8 + CH], ps[:, :], AF.Identity,
                                             bias=bqkv_sb[:, m:m + 1], scale=1.0 / S_QKV)
                    return qkvT8

                def rest_stage(c, qkvT8):
                    csl = slice(c * CH, (c + 1) * CH)
                    ad18 = p2a.tile([P, 4 * CH], FP8, tag="ad18", bufs=1)
                    ad1t = p2a.tile([P, CH], BF16, tag="ad1t", bufs=1)
                    for m in range(5):
                        ps = ps2a.tile([P, CH], FP32, tag="mm")
                        for kp in range(9):
                            lhsT = _ap(a1_sb, kp * 2 * ADP + m * P, [[ADP, 2], [1, P]])
                            rhs = _ap(qkvT8, kp * 2 * CH, [[CH, 2], [1, CH]])
                            nc.tensor.matmul(ps[:, :], lhsT, rhs, start=(kp == 0),
                                             stop=(kp == 8), perf_mode=DR)
                        if m < 4:
                            c8 = (m // 2) * 2 * CH + (m % 2) * CH
                            nc.scalar.activation(ad18[:, c8:c8 + CH], ps[:, :], AF.Gelu,
                                                 bias=ba1_sb[:, m:m + 1], scale=1.0 / S_A1)
                        else:
                            nc.scalar.activation(ad1t[:, :], ps[:, :], AF.Gelu,
                                                 bias=ba1_sb[:, m:m + 1], scale=1.0 / S_A1)
                    fin_v = []
                    for m in range(18):
                        ps = ps2a.tile([P, CH], FP32, tag="mm")
                        for kp in range(2):
                            lhsT = _ap(a2_sb, kp * 4608 + m * P, [[2304, 2], [1, P]])
                            rhs = _ap(ad18, kp * 2 * CH, [[CH, 2], [1, CH]])
                            nc.tensor.matmul(ps[:, :], lhsT, rhs, start=(kp == 0),
                                             stop=False, perf_mode=DR)
                        nc.tensor.matmul(ps[:, :], a2t_sb[:, m * P:(m + 1) * P],
                                         ad1t[:, :], start=False, stop=True)
                        tmp = p2a.tile([P, CH], BF16, tag="tmp", bufs=3)
                        nc.vector.tensor_scalar(tmp[:, :], ps[:, :], 1.0 / S_A2,
                                                ba2_sb[:, m:m + 1], op0=ALU.mult,
                                                op1=ALU.add)
                        fin = p2a.tile([P, CH], BF16, tag="fin", bufs=7,
                                       name=f"fin{c}_{m}")
                        c8 = (m // 2) * 2 * CH + (m % 2) * CH
                        nc.vector.tensor_tensor(fin[:, :], tmp[:, :],
                                                qkvT8[:, c8:c8 + CH], op=ALU.add)
                        if m < 6:
                            nc.scalar.dma_start(out=q_d[m * P:(m + 1) * P, csl], in_=fin[:, :])
                        elif m < 12:
                            nc.scalar.dma_start(out=k_d[(m - 6) * P:(m - 6 + 1) * P, csl],
                                                in_=fin[:, :])
                        else:
                            fin_v.append(fin)
                    # v transposes -> v8_d token-major fp8 (65 cols/head: v|1)
                    for tt in range(4):
                        vt_ps = pst2a.tile([P, DIM], BF16, tag="vtp")
                        for mv in range(6):
                            nc.tensor.transpose(vt_ps[:, mv * P:(mv + 1) * P],
                                                fin_v[mv][:, tt * P:(tt + 1) * P],
                                                ident[:, :])
                        v8 = p2a.tile([P, 780], FP8, tag="v8", bufs=2)
                        for ft in range(6):
                            vdst = _ap(v8, ft * 130, [[65, 2], [1, 64]])
                            vsrc = _ap(vt_ps, ft * P, [[64, 2], [1, 64]])
                            nc.scalar.copy(vdst, vsrc)
                        vones = _ap(v8, 64, [[65, NH], [1, 1]])
                        nc.vector.memset(vones, 1.0)
                        nc.scalar.dma_start(
                            out=v8_d[c * CH + tt * P: c * CH + (tt + 1) * P, :],
                            in_=v8[:, :])
                    # shuffle adapter (feature-major) + transposes -> ad_d
                    adf = []
                    for pt in range(3):
                        a0 = p2a.tile([P, CH], BF16, tag="a0", bufs=3, name=f"a0_{c}_{pt}")
                        nc.vector.tensor_scalar(a0[:, :], normT[pt][:, csl], g0c[:, pt:pt + 1],
                                                h0c[:, pt:pt + 1], op0=ALU.mult, op1=ALU.add)
                        adf.append(a0)
                    for pt in range(3):
                        s1t = p2a.tile([P, CH], BF16, tag="s1", bufs=2)
                        nc.scalar.activation(s1t[:, :], normT[pt + 3][:, csl], AF.Sigmoid,
                                             bias=Qc[:, pt:pt + 1], scale=Pc[:, pt:pt + 1])
                        t1 = p2a.tile([P, CH], BF16, tag="t1", bufs=2)
                        nc.vector.tensor_scalar(t1[:, :], normT[pt + 3][:, csl],
                                                w1c_sb[:, 3 + pt:4 + pt],
                                                b1c_sb[:, 3 + pt:4 + pt],
                                                op0=ALU.mult, op1=ALU.add)
                        xs = p2a.tile([P, CH], BF16, tag="xs", bufs=3, name=f"xs_{c}_{pt}")
                        nc.vector.tensor_tensor(xs[:, :], t1[:, :], s1t[:, :], op=ALU.mult)
                        adf.append(xs)
                    for tt in range(4):
                        adT_ps = pst2a.tile([P, DIM], BF16, tag="adp")
                        for ft in range(6):
                            nc.tensor.transpose(adT_ps[:, ft * P:(ft + 1) * P],
                                                adf[ft][:, tt * P:(tt + 1) * P],
                                                ident[:, :])
                        ad_sb = p2a.tile([P, DIM], BF16, tag="adsb", bufs=2)
                        for ft in range(6):
                            coff = ft * 256 if ft < 3 else (ft - 3) * 256 + 1
                            dst = _ap(ad_sb, coff, [[2, P]])
                            nc.scalar.copy(dst, adT_ps[:, ft * P:(ft + 1) * P])
                        nc.scalar.dma_start(
                            out=ad_d[c * CH + tt * P: c * CH + (tt + 1) * P, :],
                            in_=ad_sb[:, :])

                pend_q8 = None
                for c in range(NCH):
                    cur = qkv_stage(c)
                    if pend_q8 is not None:
                        rest_stage(c - 1, pend_q8)
                    pend_q8 = cur
                rest_stage(NCH - 1, pend_q8)

            nrm_ctx.__exit__(None, None, None)
            n2t_ctx = tc.tile_pool(name="n2t", bufs=1)
            n2tp = n2t_ctx.__enter__()
            norm2T = [n2tp.tile([P, T], BF16, name=f"norm2T{k}") for k in range(6)]

            # ============ PHASE 2b: windowed attention ============
            with tc.tile_pool(name="w2b", bufs=1) as w2b, \
                 tc.tile_pool(name="p2b", bufs=2) as p2b, \
                 tc.tile_pool(name="psM", bufs=2, space="PSUM") as psM, \
                 tc.tile_pool(name="psS", bufs=2, space="PSUM") as psS, \
                 tc.tile_pool(name="psT", bufs=1, space="PSUM") as psT:
                wp_sb = w2b.tile([P, 6 * DIM], FP8)
                for k in range(6):
                    o0 = (k // 2) * 2 * DIM + (k % 2) * DIM
                    nc.scalar.dma_start(out=wp_sb[:, o0:o0 + DIM],
                                        in_=wp8[k * P:(k + 1) * P, :])
                def scores_stage(w):
                    q_sb = p2b.tile([HD, NH * NT], BF16, tag="q", bufs=3)
                    gsrc = bass.AP(tensor=q_d[:, :].tensor, offset=w * NT,
                                   ap=[[T, HD], [HD * T, NH], [1, NT]])
                    nc.sync.dma_start(out=q_sb[:, :], in_=gsrc)
                    k_sb = p2b.tile([HD, NH * NT], BF16, tag="k", bufs=3)
                    gsrc = bass.AP(tensor=k_d[:, :].tensor, offset=w * NT,
                                   ap=[[T, HD], [HD * T, NH], [1, NT]])
                    nc.sync.dma_start(out=k_sb[:, :], in_=gsrc)
                    v8_sb = p2b.tile([P, 2 * 780], FP8, tag="v8", bufs=3)
                    gsrc = bass.AP(tensor=v8_d[:, :].tensor, offset=w * NT * 780,
                                   ap=[[780, P], [P * 780, 2], [1, 780]])
                    nc.sync.dma_start(out=v8_sb[:, :], in_=gsrc)
                    # rel-pos stages, whole window, merged q-ordered layout:
                    #   esb[j, h*256 + q] with q = G*16 + t; H rows 0..31, W rows 32..47
                    esb = p2b.tile([48, NH * NT], BF16, tag="esb", bufs=2)
                    for Gp in range(8):
                        e_ps = psS.tile([P, 512], FP32, tag="S")
                        for gi in range(2):
                            G = Gp * 2 + gi
                            nc.tensor.matmul(e_ps[0:32, gi * 192:(gi + 1) * 192],
                                             rh_sb[:, G * 32:(G + 1) * 32],
                                             _ap(q_sb, G * 16, [[NT, NH], [1, 16]]),
                                             start=True, stop=True)
                        hdst = bass.AP(tensor=esb.tensor,
                                       offset=esb[0:32, :].offset + Gp * 32,
                                       ap=[[esb.tensor.shape[1], 32], [16, 2],
                                           [NT, NH], [1, 16]])
                        hsrc = bass.AP(tensor=e_ps.tensor,
                                       offset=e_ps[0:32, :].offset,
                                       ap=[[e_ps.tensor.shape[1], 32], [192, 2],
                                           [16, NH], [1, 16]])
                        nc.scalar.copy(hdst, hsrc)
                    for wp_ in range(8):
                        e_ps = psS.tile([P, 512], FP32, tag="S")
                        for wi in range(2):
                            wg = wp_ * 2 + wi
                            nc.tensor.matmul(e_ps[32:48, wi * 192:(wi + 1) * 192],
                                             rw_sb[:, wg * 16:(wg + 1) * 16],
                                             _ap(q_sb, wg, [[NT, NH], [16, 16]]),
                                             start=True, stop=True,
                                             tile_position=(0, 32))
                        wdst = bass.AP(tensor=esb.tensor,
                                       offset=esb[32:48, :].offset + wp_ * 2,
                                       ap=[[esb.tensor.shape[1], 16], [1, 2],
                                           [NT, NH], [16, 16]])
                        wsrc = bass.AP(tensor=e_ps.tensor,
                                       offset=e_ps[32:48, :].offset,
                                       ap=[[e_ps.tensor.shape[1], 16], [192, 2],
                                           [16, NH], [1, 16]])
                        nc.scalar.copy(wdst, wsrc)
                    # S^T + exp -> pT8 [128, (2kc, 12h, 256q)]
                    pT8 = p2b.tile([P, 2 * NH * NT], FP8, tag="pT8", bufs=2)
                    for kc in range(2):
                        for hp in range(6):
                            s_ps = psS.tile([P, 512], FP32, tag="S")
                            for hh in range(2):
                                h = hp * 2 + hh
                                nc.tensor.matmul(
                                    s_ps[:, hh * NT:(hh + 1) * NT],
                                    k_sb[:, h * NT + kc * P: h * NT + (kc + 1) * P],
                                    q_sb[:, h * NT:(h + 1) * NT],
                                    start=True, stop=False)
                                nc.tensor.matmul(
                                    s_ps[:, hh * NT:(hh + 1) * NT],
                                    ind48_sb[:, kc * P:(kc + 1) * P],
                                    esb[:, h * NT:(h + 1) * NT],
                                    start=False, stop=True)
                            o0 = kc * 3072 + hp * 512
                            nc.scalar.activation(pT8[:, o0:o0 + 512], s_ps[:, :], AF.Exp)
                    return v8_sb, pT8

                def consume_stage(w, v8_sb, pT8):
                    for qt in range(2):
                        tglob = w * 2 + qt
                        # PV with ones-augmented v8 (softmax sums fused)
                        oAB = []
                        for hb in range(2):
                            o_ps = psS.tile([P, 512], FP32, tag="S")
                            for hh in range(6):
                                h = hb * 6 + hh
                                lhsT = _ap(pT8, h * NT + qt * P, [[3072, 2], [1, P]])
                                nc.tensor.matmul(o_ps[:, hh * 65:hh * 65 + 65], lhsT,
                                                 _ap(v8_sb, h * 65, [[780, 2], [1, 65]]),
                                                 start=True, stop=True, perf_mode=DR)
                            oAB.append(o_ps)
                        rec = p2b.tile([P, NH], FP32, tag="rec", bufs=2)
                        for hb in range(2):
                            rsrc = bass.AP(tensor=oAB[hb].tensor,
                                           offset=oAB[hb][:, :].offset + 64,
                                           ap=[[oAB[hb].tensor.shape[1], P], [65, 6]])
                            nc.vector.reciprocal(rec[:, hb * 6:(hb + 1) * 6], rsrc)
                        attn_sb = p2b.tile([P, DIM], BF16, tag="attn", bufs=2)
                        for h in range(NH):
                            hs = (h % 6) * 65
                            nc.vector.tensor_scalar(attn_sb[:, h * HD:(h + 1) * HD],
                                                    oAB[h // 6][:, hs:hs + 64],
                                                    rec[:, h:h + 1], None, op0=ALU.mult)
                        # oT + proj
                        oT_ps = psT.tile([P, DIM], BF16, tag="T")
                        for kt in range(6):
                            nc.tensor.transpose(oT_ps[:, kt * P:(kt + 1) * P],
                                                attn_sb[:, kt * P:(kt + 1) * P],
                                                ident[:, :])
                        oT8 = p2b.tile([P, DIM], FP8, tag="oT8", bufs=2)
                        for kt in range(6):
                            c8 = (kt // 2) * 2 * P + (kt % 2) * P
                            nc.scalar.copy(oT8[:, c8:c8 + P], oT_ps[:, kt * P:(kt + 1) * P])
                        pr_ps = psM.tile([P, 784], FP32, tag="M")
                        for c0, cn in ((0, 512), (512, 256)):
                            for kp in range(3):
                                lhsT = _ap(oT8, kp * 2 * P, [[P, 2], [1, P]])
                                rhs = _ap(wp_sb, kp * 2 * DIM + c0, [[DIM, 2], [1, cn]])
                                nc.tensor.matmul(pr_ps[:, c0:c0 + cn], lhsT, rhs,
                                                 start=(kp == 0), stop=False, perf_mode=DR)
                            nc.tensor.matmul(pr_ps[:, c0:c0 + cn], ones1[:, :],
                                             bp_sb[:, c0:c0 + cn], start=False, stop=True)
                        # residual + LN2 + norm2T
                        tsl = slice(tglob * P, (tglob + 1) * P)
                        xt = p2b.tile([P, DIM], FP32, tag="xres", bufs=3)
                        nc.sync.dma_start(out=xt[:, :], in_=x_in[tsl, :])
                        adt = p2b.tile([P, DIM], BF16, tag="adt", bufs=3)
                        nc.sync.dma_start(out=adt[:, :], in_=ad_d[tsl, :])
                        tmp = p2b.tile([P, DIM], FP32, tag="tmpr", bufs=2)
                        nc.vector.scalar_tensor_tensor(tmp[:, :], adt[:, :], BLOCK_SCALE,
                                                       xt[:, :], op0=ALU.mult, op1=ALU.add)
                        xm = p2b.tile([P, DIM], BF16, tag="xm", bufs=2)
                        sm2 = p2b.tile([P, 1], FP32, tag="sm2", bufs=2)
                        nc.vector.scalar_tensor_tensor(xm[:, :], pr_ps[:, 0:DIM],
                                                       1.0 / S_WP, tmp[:, :],
                                                       op0=ALU.mult, op1=ALU.add,
                                                       accum_out=sm2[:, :])
                        nc.scalar.dma_start(out=xm_d[tsl, :], in_=xm[:, :])
                        scr2 = p2b.tile([P, DIM], BF16, tag="scr2", bufs=2)
                        sq2 = p2b.tile([P, 1], FP32, tag="sq2", bufs=2)
                        nc.scalar.activation(scr2[:, :], xm[:, :], AF.Square,
                                             accum_out=sq2[:, :])
                        m2 = p2b.tile([P, 1], FP32, tag="m2", bufs=2)
                        nc.vector.tensor_scalar(m2[:, :], sm2[:, :], 1.0 / DIM, None, op0=ALU.mult)
                        v2 = p2b.tile([P, 1], FP32, tag="v2", bufs=2)
                        nc.vector.tensor_scalar(v2[:, :], sq2[:, :], 1.0 / DIM, None, op0=ALU.mult)
                        m2sq = p2b.tile([P, 1], FP32, tag="m2sq", bufs=2)
                        nc.vector.tensor_tensor(m2sq[:, :], m2[:, :], m2[:, :], op=ALU.mult)
                        nc.vector.tensor_tensor(v2[:, :], v2[:, :], m2sq[:, :], op=ALU.subtract)
                        sd2 = p2b.tile([P, 1], FP32, tag="sd2", bufs=2)
                        nc.scalar.activation(sd2[:, :], v2[:, :], AF.Sqrt, bias=eps_col[:, :])
                        r2 = p2b.tile([P, 1], FP32, tag="r2", bufs=2)
                        nc.vector.reciprocal(r2[:, :], sd2[:, :])
                        n2 = p2b.tile([P, DIM], BF16, tag="n2", bufs=2)
                        nc.vector.tensor_scalar(n2[:, :], xm[:, :], m2[:, :], r2[:, :],
                                                op0=ALU.subtract, op1=ALU.mult)
                        n2T_ps = psT.tile([P, DIM], BF16, tag="T")
                        for kt in range(6):
                            nc.tensor.transpose(n2T_ps[:, kt * P:(kt + 1) * P],
                                                n2[:, kt * P:(kt + 1) * P], ident[:, :])
                        for kt in range(6):
                            nc.scalar.copy(norm2T[kt][:, tglob * P:(tglob + 1) * P],
                                           n2T_ps[:, kt * P:(kt + 1) * P])

                pend_w = None
                for w in range(NWIN):
                    cur = scores_stage(w)
                    if pend_w is not None:
                        consume_stage(w - 1, pend_w[0], pend_w[1])
                    pend_w = cur
                consume_stage(NWIN - 1, pend_w[0], pend_w[1])

            # ============ PHASE 5: MLP ============
            with tc.tile_pool(name="w5", bufs=1) as w5, \
                 tc.tile_pool(name="p5", bufs=2) as p5, \
                 tc.tile_pool(name="h5", bufs=25) as h5, \
                 tc.tile_pool(name="ps5", bufs=3, space="PSUM") as ps5, \
                 tc.tile_pool(name="ps5b", bufs=2, space="PSUM") as ps5b:
                wm1_sb = [w5.tile([P, MLPD], BF16, tag="wm1", bufs=6, name=f"wm1_{_i}")
                          for _i in range(6)]
                for k in range(6):
                    nc.scalar.dma_start(out=wm1_sb[k][:, :], in_=wm1[k * P:(k + 1) * P, :])
                wm2_sb = [w5.tile([P, DIM], BF16, tag="wm2", bufs=24, name=f"wm2_{_i}")
                          for _i in range(24)]
                for k in range(24):
                    nc.scalar.dma_start(out=wm2_sb[k][:, :], in_=wm2[k * P:(k + 1) * P, :])
                for c in range(NCH):
                    csl = slice(c * CH, (c + 1) * CH)
                    hT = [h5.tile([P, CH], BF16, tag="hT", bufs=25, name=f"hT{c}_{_i}")
                          for _i in range(24)]
                    for m in range(24):
                        ps = ps5.tile([P, CH], FP32, tag="mm", bufs=3)
                        for k in range(6):
                            nc.tensor.matmul(ps[:, :], wm1_sb[k][:, m * P:(m + 1) * P],
                                             norm2T[k][:, csl], start=(k == 0), stop=(k == 5))
                        nc.scalar.activation(hT[m][:, :], ps[:, :], AF.Gelu,
                                             bias=bm1_sb[:, m:m + 1])
                    for tt in range(CH // P):
                        tglob = c * (CH // P) + tt
                        ps = ps5b.tile([P, DIM], FP32, tag="mm2", bufs=2)
                        for n2_, nsl in ((0, slice(0, 512)), (1, slice(512, 768))):
                            for k in range(24):
                                nc.tensor.matmul(ps[:, nsl],
                                                 hT[k][:, tt * P:(tt + 1) * P],
                                                 wm2_sb[k][:, nsl],
                                                 start=(k == 0), stop=False)
                            nc.tensor.matmul(ps[:, nsl], ones1[:, :], bm2_sb[:, nsl],
                                             start=False, stop=True)
                        xm_t = p5.tile([P, DIM], BF16, tag="xmt", bufs=3)
                        nc.sync.dma_start(out=xm_t[:, :],
                                          in_=xm_d[tglob * P:(tglob + 1) * P, :])
                        ot = p5.tile([P, DIM], FP32, tag="ot", bufs=3)
                        nc.vector.tensor_tensor(ot[:, :], ps[:, :], xm_t[:, :], op=ALU.add)
                        nc.scalar.dma_start(out=out_ext[tglob * P:(tglob + 1) * P, :],
                                            in_=ot[:, :])
            n2t_ctx.__exit__(None, None, None)

    nc.finalize()
    return nc


_GRAPH = None
_SCALES = None


def _window_permute(x):
    xb = x.reshape(B, H // WS, WS, W // WS, WS, DIM).transpose(0, 1, 3, 2, 4, 5)
    return np.ascontiguousarray(xb.reshape(B, T, DIM))


def _window_unpermute(y):
    yb = y.reshape(B, H // WS, W // WS, WS, WS, DIM).transpose(0, 1, 3, 2, 4, 5)
    return np.ascontiguousarray(yb.reshape(B, H, W, DIM))


def make_feeds(w1, b1, Wqkv, bqkv, A1, ba1, A2, ba2, aw, rel_h, rel_w, Wp, bp,
               cw, cb, sw, sb, w2, b2, Wm1, bm1, Wm2, bm2):
    """Host-side weight folding; returns (feeds, scales)."""
    Wqkv_f = w1[:, None] * Wqkv
    bqkv_f = b1 @ Wqkv + bqkv
    ksl = slice(DIM, 2 * DIM)
    Wqkv_f[:, ksl] *= SCALE
    bqkv_k = bqkv_f.copy(); bqkv_k[ksl] *= SCALE
    A1_f = A1.copy(); A1_f[ksl, :] /= SCALE
    A1_p = np.zeros((3 * DIM, ADP), np.float32); A1_p[:, :AD] = A1_f
    ba1_p = np.zeros(ADP, np.float32); ba1_p[:AD] = ba1
    A2_f = aw * A2
    ba2_f = aw * ba2
    A2_f[:, ksl] *= SCALE
    ba2_k = ba2_f.copy(); ba2_k[ksl] *= SCALE
    A2_p = np.zeros((ADP, 3 * DIM), np.float32); A2_p[:AD, :] = A2_f
    Wm1_f = w2[:, None] * Wm1
    bm1_f = b2 @ Wm1 + bm1

    S_QKV = _pow2_scale(Wqkv_f)
    S_A1 = _pow2_scale(A1_p)
    S_A2 = _pow2_scale(A2_p)
    S_WP = _pow2_scale(Wp)
    scales = (S_QKV, S_A1, S_A2, S_WP)

    # rel-pos matrices: RHcat[c, G*32+j] = rel_h[G-j+15, c] (j<16)
    rhcat = np.zeros((HD, 512), np.float32)
    rwcat = np.zeros((HD, 256), np.float32)
    for G in range(16):
        for j in range(16):
            rhcat[:, G * 32 + j] = rel_h[G - j + 15, :]
            rwcat[:, G * 16 + j] = rel_w[G - j + 15, :]
    indic48 = np.zeros((48, NT), np.float32)
    for j in range(16):
        for kw in range(16):
            indic48[j, j * 16 + kw] = 1.0       # kh one-hot
            indic48[32 + j, kw * 16 + j] = 1.0  # kw one-hot
    feeds = {
        "wqkv8": _fp8(Wqkv_f, S_QKV),
        "a1w8": _fp8(A1_p, S_A1),
        "a2w8": _fp8(A2_p[:512], S_A2),
        "a2wt": _bf16(A2_p[512:640] * S_A2),
        "wp8": _fp8(Wp, S_WP),
        "wm1": _bf16(Wm1_f), "wm2": _bf16(Wm2),
        "rhcat": _bf16(rhcat), "rwcat": _bf16(rwcat),
        "indic48": _bf16(indic48),
        "bqkv_c": _col_tiles(bqkv_k), "ba1_c": _col_tiles(ba1_p),
        "ba2_c": _col_tiles(ba2_k), "bm1_c": _col_tiles(bm1_f),
        "bp_r": _bf16(bp.reshape(1, DIM) * S_WP),
        "bm2_r": _bf16(bm2.reshape(1, DIM)),
        "w1_c": _col_tiles(w1), "b1_c": _col_tiles(b1),
        "w1_r": w1.reshape(1, DIM).astype(np.float32),
        "b1_r": b1.reshape(1, DIM).astype(np.float32),
        "cw_r": cw.reshape(1, HID).astype(np.float32),
        "cb_r": cb.reshape(1, HID).astype(np.float32),
        "sw_r": sw.reshape(1, HID).astype(np.float32),
        "sb_r": sb.reshape(1, HID).astype(np.float32),
    }
    return feeds, scales


def kernel(x, w1, b1, Wqkv, bqkv, A1, ba1, A2, ba2, aw, rel_h, rel_w, Wp, bp,
           cw, cb, sw, sb, w2, b2, Wm1, bm1, Wm2, bm2):
    global _GRAPH, _SCALES
    x = np.asarray(x, np.float32)
    f = lambda a: np.asarray(a, np.float32)
    feeds, scales = make_feeds(
        f(w1), f(b1), f(Wqkv), f(bqkv), f(A1), f(ba1), f(A2), f(ba2),
        float(np.asarray(aw)), f(rel_h), f(rel_w), f(Wp), f(bp),
        f(cw).ravel(), f(cb).ravel(), f(sw).ravel(), f(sb).ravel(),
        f(w2), f(b2), f(Wm1), f(bm1), f(Wm2), f(bm2))

    xp = _window_permute(x)
    in_maps = [dict(feeds, x=np.ascontiguousarray(xp[i])) for i in range(B)]

    if _GRAPH is None or _SCALES != scales:
        _GRAPH = build_graph(scales)
        _SCALES = scales
    import os
    trace = os.environ.get("KTRACE", "0") == "1"
    kw = {}
    if os.environ.get("KTMPDIR"):
        kw["tmpdir"] = os.environ["KTMPDIR"]
    res = run_bass_kernel_spmd(_GRAPH, in_maps, core_ids=list(range(B)),
                               trace=trace, **kw)
    if trace and res.exec_time_ns is not None:
        print(f"HW exec time: {res.exec_time_ns} ns")
    y = np.stack([res.results[i]["out"] for i in range(B)], 0)
    return _window_unpermute(y).astype(np.float32)
